# revision 7
# baseline (speedup 1.0000x reference)
"""Trainium2 Bass kernel for nn_DendriticAttentionLayer.

Math (per batch row b):
  q,k,v = x@W{q,k,v}.T + b;  per-head logit = (beta_h * sum_d v_heads + (1-beta_h)
  * sum_d q*k) / sqrt(D) / temp;  attn = softmax over H;  combined = attn * v;
  out = (combined * sigmoid(x@Wg.T+bg)) @ Wo.T + bo;
  v_out_new = ba*v_out + (1-ba)*out;  spike = v_out_new > 1;  v_final = v_out_new - spike.

Strategy: data-parallel across 8 cores (512 batch rows each), weights replicated.
Everything on-chip lives in a transposed [feature, batch] layout so no transposes
are needed: the five big GEMMs compute out.T[j,b] = sum_c W[j,c] * x.T[c,b] with
128x128 weight chunks as the PE stationary operand (streamed from HBM in bf16) and
the resident x.T (later cg.T) as the moving operand; fp32 PSUM accumulation.
Per-head logit sums (a partition-dim reduction) are tiny indicator matmuls into a
persistent PSUM tile; softmax-over-heads uses ones-matmul partition broadcasts.
v_heads only feeds the logit sums, so its contribution is folded in on the host.
"""

from contextlib import ExitStack

import ml_dtypes
import numpy as np

import concourse.tile as tile
from concourse import bacc, mybir
from concourse.bass_utils import run_bass_kernel_spmd

P = 128
N_CORES = 8
B, C, H = 4096, 4096, 16
TAU_MIN, TAU_MAX = 2.0, 16.0
V_TH = 1.0
BF16 = mybir.dt.bfloat16
F32 = mybir.dt.float32
NP_BF16 = ml_dtypes.bfloat16
Act = mybir.ActivationFunctionType
Alu = mybir.AluOpType


def _betas(h):
    return (1.0 - 1.0 / np.linspace(TAU_MIN, TAU_MAX, h)).astype(np.float32)


def build_nc(b_loc=B // N_CORES, c=C, h=H):
    """Build the per-core kernel. Returns finalized Bacc instance."""
    nch = c // P  # number of 128-wide feature chunks
    d = c // h
    cph = d // P  # chunks per head
    assert d % P == 0 and c % P == 0
    one_m_ba = float(np.float32(1.0) - np.float32(_betas(h).mean()))

    nc = bacc.Bacc("TRN2", target_bir_lowering=False, debug=False)

    def din(name, shape, dt=BF16):
        return nc.dram_tensor(name, shape, dt, kind="ExternalInput").ap()

    x_d = din("xT", [P, nch, b_loc])
    w_d = {k: din("w" + k, [nch, P, nch, P]) for k in "kqvgo"}
    bias_d = {k: din("b" + k, [P, nch], F32) for k in "kqvg"}
    ind_d = din("ind", [P, nch, h])
    sel_d = din("sel", [h, nch, P])
    onesf_d = din("onesf", [h, h], F32)
    coef_d = din("coef", [h, 1], F32)
    vhpre_d = din("vhpre", [h, b_loc], F32)
    vpre_d = din("vpre", [nch, P, b_loc], F32)
    spike_d = nc.dram_tensor("spikeT", [nch, P, b_loc], F32, kind="ExternalOutput").ap()
    vout_d = nc.dram_tensor("voutT", [nch, P, b_loc], F32, kind="ExternalOutput").ap()

    with tile.TileContext(nc) as tc, ExitStack() as ctx:
        const = ctx.enter_context(tc.tile_pool(name="const", bufs=1))
        bigbuf = ctx.enter_context(tc.tile_pool(name="bigbuf", bufs=2))
        wpool = ctx.enter_context(tc.tile_pool(name="wpool", bufs=2))
        evac = ctx.enter_context(tc.tile_pool(name="evac", bufs=3))
        vprep = ctx.enter_context(tc.tile_pool(name="vprep", bufs=3))
        outp = ctx.enter_context(tc.tile_pool(name="outp", bufs=2))
        ps_big = ctx.enter_context(tc.tile_pool(name="ps_big", bufs=3, space="PSUM"))
        ps_logit = ctx.enter_context(tc.tile_pool(name="ps_logit", bufs=1, space="PSUM"))
        ps_bc = ctx.enter_context(tc.tile_pool(name="ps_bc", bufs=2, space="PSUM"))
        ps_sm = ctx.enter_context(tc.tile_pool(name="ps_sm", bufs=1, space="PSUM"))

        # ---- resident constants ----
        xT = const.tile([P, nch, b_loc], BF16)
        nc.sync.dma_start(xT[:], x_d)
        bias_sb = {}
        for k in "kqvg":
            t = const.tile([P, nch], F32, tag="bias_" + k)
            nc.sync.dma_start(t[:], bias_d[k])
            bias_sb[k] = t
        ind_sb = const.tile([P, nch, h], BF16)
        nc.sync.dma_start(ind_sb[:], ind_d)
        sel_sb = const.tile([h, nch, P], BF16)
        nc.sync.dma_start(sel_sb[:], sel_d)
        onesf = const.tile([h, h], F32)
        nc.sync.dma_start(onesf[:], onesf_d)
        coef = const.tile([h, 1], F32)
        nc.sync.dma_start(coef[:], coef_d)
        vhpre = const.tile([h, b_loc], F32)
        nc.sync.dma_start(vhpre[:], vhpre_d)

        k_full = bigbuf.tile([P, nch, b_loc], BF16, tag="big")
        v_full = const.tile([P, nch, b_loc], BF16)

        def gemm_pass(w_ap, rhs, consume, post_chunk=None):
            """out.T[jc] = sum_co w[jc,:,co,:].T @ rhs[:,co,:], consume(jc, psum).
            consume for chunk jc is emitted after chunk jc+1's matmuls so the PE
            never waits on an evacuation chain."""
            pending = None
            for jc in range(nch):
                wt = wpool.tile([P, nch, P], BF16, tag="w")
                nc.sync.dma_start(wt[:], w_ap[jc])
                ps = ps_big.tile([P, b_loc], F32, tag="ps")
                for co in range(nch):
                    nc.tensor.matmul(
                        ps[:],
                        wt[:, co, :],
                        rhs[:, co, :],
                        start=(co == 0),
                        stop=(co == nch - 1),
                    )
                if pending is not None:
                    pending()
                if post_chunk is not None:
                    post_chunk(jc)

                def _consume(jc=jc, ps=ps):
                    consume(jc, ps)

                pending = _consume
            pending()

        # ---- K pass ----
        def consume_kv(dest, bkey):
            def f(jc, ps):
                nc.scalar.add(dest[:, jc, :], ps[:], bias_sb[bkey][:, jc : jc + 1])

            return f

        gemm_pass(w_d["k"], xT, consume_kv(k_full, "k"))

        # ---- Q pass: fused q*k product and per-head logit accumulation ----
        logits_ps = ps_logit.tile([h, b_loc], F32)

        def consume_q(jc, ps):
            q_sb = evac.tile([P, b_loc], BF16, tag="q")
            nc.scalar.add(q_sb[:], ps[:], bias_sb["q"][:, jc : jc + 1])
            qk = evac.tile([P, b_loc], BF16, tag="qk")
            nc.vector.tensor_mul(qk[:], q_sb[:], k_full[:, jc, :])
            nc.tensor.matmul(
                logits_ps[:],
                ind_sb[:, jc, :],
                qk[:],
                start=(jc == 0),
                stop=(jc == nch - 1),
            )

        gemm_pass(w_d["q"], xT, consume_q)

        # ---- softmax over heads (emitted inside the V pass to keep PE fed) ----
        logits_sb = const.tile([h, b_loc], F32)
        expT = const.tile([h, b_loc], F32)
        sum_sb = const.tile([1, b_loc], F32)
        recip = const.tile([h, b_loc], F32)
        attn_bf = const.tile([h, b_loc], BF16)

        def softmax_a():
            # logits = coef_h * qk_sums + vh_pre ; exp ; sum over heads (PE)
            nc.vector.tensor_scalar(
                logits_sb[:], logits_ps[:], coef[:, 0:1], None, op0=Alu.mult
            )
            nc.vector.tensor_add(logits_sb[:], logits_sb[:], vhpre[:])
            nc.scalar.activation(expT[:], logits_sb[:], Act.Exp)
            sum_ps = ps_sm.tile([1, b_loc], F32, tag="sm1")
            nc.tensor.matmul(sum_ps[:], onesf[:, 0:1], expT[:], start=True, stop=True)
            nc.scalar.copy(sum_sb[:], sum_ps[:])

        def softmax_b():
            sumb_ps = ps_sm.tile([h, b_loc], F32, tag="sm2")
            nc.tensor.matmul(
                sumb_ps[:], onesf[0:1, :], sum_sb[:], start=True, stop=True
            )
            nc.vector.reciprocal(recip[:], sumb_ps[:])
            nc.vector.tensor_mul(attn_bf[:], expT[:], recip[:])

        def post_v(jc):
            if jc == 0:
                softmax_a()
            elif jc == 1:
                softmax_b()

        # ---- V pass ----
        gemm_pass(w_d["v"], xT, consume_kv(v_full, "v"), post_chunk=post_v)

        # ---- G pass: sigmoid gate, attn broadcast, cg = attn*v*gate ----
        cg_full = bigbuf.tile([P, nch, b_loc], BF16, tag="big")

        def consume_g(jc, ps):
            gate = evac.tile([P, b_loc], BF16, tag="gate")
            nc.scalar.activation(
                gate[:], ps[:], Act.Sigmoid, bias=bias_sb["g"][:, jc : jc + 1]
            )
            bc = ps_bc.tile([P, b_loc], F32, tag="bc")
            nc.tensor.matmul(
                bc[:], sel_sb[:, jc, :], attn_bf[:], start=True, stop=True
            )
            nc.vector.tensor_mul(cg_full[:, jc, :], v_full[:, jc, :], bc[:])
            nc.vector.tensor_mul(cg_full[:, jc, :], cg_full[:, jc, :], gate[:])

        gemm_pass(w_d["g"], xT, consume_g)

        # ---- O pass: out.T, then leaky-integrate + spike ----
        def consume_o(jc, ps):
            vp = vprep.tile([P, b_loc], F32, tag="vp")
            nc.sync.dma_start(vp[:], vpre_d[jc])
            vnew = outp.tile([P, b_loc], F32, tag="vnew")
            nc.scalar.mul(vnew[:], ps[:], one_m_ba)
            nc.vector.tensor_add(vnew[:], vnew[:], vp[:])
            spike = outp.tile([P, b_loc], F32, tag="spike")
            nc.vector.tensor_scalar(spike[:], vnew[:], float(V_TH), None, op0=Alu.is_gt)
            nc.gpsimd.dma_start(spike_d[jc], spike[:])
            nc.vector.tensor_sub(vnew[:], vnew[:], spike[:])
            nc.gpsimd.dma_start(vout_d[jc], vnew[:])

        gemm_pass(w_d["o"], cg_full, consume_o)

    nc.finalize()
    return nc


# ---------------------------------------------------------------------------
# host-side prep + dispatch
# ---------------------------------------------------------------------------

_NC_CACHE = {}


def _get_nc(*dims):
    if dims not in _NC_CACHE:
        _NC_CACHE[dims] = build_nc(*dims)
    return _NC_CACHE[dims]


def _prep_w(W, nch):
    # A[jc, ci, co, jj] = W[jc*128+jj, co*128+ci]
    return np.ascontiguousarray(
        W.astype(NP_BF16).reshape(nch, P, nch, P).transpose(0, 3, 2, 1)
    )


def prep_inputs(
    x, Wq, bq, Wk, bk, Wv, bv, Wg, bg, Wo, bo, temperature, v_heads, v_out,
    n_cores=N_CORES,
):
    b, c = x.shape
    bh, h, d = v_heads.shape
    nch = c // P
    cph = d // P
    b_loc = b // n_cores
    betas = _betas(h)
    ba = np.float32(betas.mean())
    temp = np.float32(np.asarray(temperature).reshape(-1)[0])
    scal = np.float32(1.0) / (np.float32(np.sqrt(d)) * temp)

    shared = {
        "wq": _prep_w(np.asarray(Wq), nch),
        "wk": _prep_w(np.asarray(Wk), nch),
        "wv": _prep_w(np.asarray(Wv), nch),
        "wg": _prep_w(np.asarray(Wg), nch),
        "wo": _prep_w(np.asarray(Wo), nch),
        "bq": np.ascontiguousarray(np.asarray(bq, np.float32).reshape(nch, P).T),
        "bk": np.ascontiguousarray(np.asarray(bk, np.float32).reshape(nch, P).T),
        "bv": np.ascontiguousarray(np.asarray(bv, np.float32).reshape(nch, P).T),
        "bg": np.ascontiguousarray(np.asarray(bg, np.float32).reshape(nch, P).T),
        "onesf": np.ones((h, h), np.float32),
        "coef": ((1.0 - betas) * scal).astype(np.float32).reshape(h, 1),
    }
    ind = np.zeros((P, nch, h), NP_BF16)
    sel = np.zeros((h, nch, P), NP_BF16)
    for jc in range(nch):
        ind[:, jc, jc // cph] = 1.0
        sel[jc // cph, jc, :] = 1.0
    shared["ind"] = ind
    shared["sel"] = sel

    bo32 = np.asarray(bo, np.float32)
    in_maps = []
    for cid in range(n_cores):
        sl = slice(cid * b_loc, (cid + 1) * b_loc)
        xs = np.asarray(x[sl], np.float32)
        xT = np.ascontiguousarray(
            xs.reshape(b_loc, nch, P).transpose(2, 1, 0).astype(NP_BF16)
        )
        vh = np.asarray(v_heads[sl], np.float32)
        vhpre = np.ascontiguousarray(
            (betas[None, :] * vh.sum(-1) * scal).T.astype(np.float32)
        )
        vo = np.asarray(v_out[sl], np.float32)
        vpre = ba * vo + (np.float32(1.0) - ba) * bo32[None, :]
        vpreT = np.ascontiguousarray(vpre.reshape(b_loc, nch, P).transpose(1, 2, 0))
        in_maps.append(
            dict(shared, xT=xT, vhpre=vhpre, vpre=vpreT)
        )
    return in_maps, (b_loc, c, h)


_RUNNER_CACHE = {}


def _make_runner(dims, n_cores):
    """Persistent jitted SPMD executable (mirrors bass2jax.run_bass_via_pjrt's
    multi-core branch, but reusable across calls so benchmarking doesn't
    recompile). Returns run(in_maps, n_iters) -> (results, times_s)."""
    import jax
    import numpy as jnp_np  # noqa
    from jax.experimental.shard_map import shard_map
    from jax.sharding import Mesh, PartitionSpec

    from concourse import bass2jax, mybir as _mybir

    nc = _get_nc(*dims)
    bass2jax.install_neuronx_cc_hook()
    assert nc.dbg_addr is None
    partition_name = nc.partition_id_tensor.name if nc.partition_id_tensor else None

    in_names, out_names, out_avals = [], [], []
    for alloc in nc.m.functions[0].allocations:
        if not isinstance(alloc, _mybir.MemoryLocationSet):
            continue
        name = alloc.memorylocations[0].name
        if alloc.kind == "ExternalInput":
            if name != partition_name:
                in_names.append(name)
        elif alloc.kind == "ExternalOutput":
            out_names.append(name)
            out_avals.append(
                jax.core.ShapedArray(tuple(alloc.tensor_shape), _mybir.dt.np(alloc.dtype))
            )
    n_params = len(in_names)
    all_names = in_names + out_names
    if partition_name is not None:
        all_names = all_names + [partition_name]
    donate = tuple(range(n_params, n_params + len(out_names)))

    def _body(*args):
        operands = list(args)
        if partition_name is not None:
            operands.append(bass2jax.partition_id_tensor())
        outs = bass2jax._bass_exec_p.bind(
            *operands,
            out_avals=tuple(out_avals),
            in_names=tuple(all_names),
            out_names=tuple(out_names),
            lowering_input_output_aliases=(),
            sim_require_finite=True,
            sim_require_nnan=True,
            nc=nc,
        )
        return tuple(outs)

    devices = jax.devices()[:n_cores]
    mesh = Mesh(np.asarray(devices), ("core",))
    in_specs = (PartitionSpec("core"),) * (n_params + len(out_names))
    out_specs = (PartitionSpec("core"),) * len(out_names)
    sharded = jax.jit(
        shard_map(_body, mesh=mesh, in_specs=in_specs, out_specs=out_specs, check_rep=False),
        donate_argnums=donate,
        keep_unused=True,
    )
    sharding = jax.sharding.NamedSharding(mesh, PartitionSpec("core"))

    def run(in_maps, n_iters=1):
        import time as _time

        concat_in = [
            jax.device_put(
                np.concatenate([np.asarray(m[name])[None] for m in in_maps], axis=0
                               ).reshape(n_cores * np.asarray(in_maps[0][name]).shape[0],
                                         *np.asarray(in_maps[0][name]).shape[1:]),
                sharding,
            )
            for name in in_names
        ]
        jax.block_until_ready(concat_in)

        def fresh_zeros():
            z = [
                jax.device_put(
                    np.zeros((n_cores * av.shape[0], *av.shape[1:]), av.dtype), sharding
                )
                for av in out_avals
            ]
            jax.block_until_ready(z)
            return z

        times = []
        out_arrs = None
        for _ in range(n_iters):
            zeros = fresh_zeros()
            t0 = _time.perf_counter()
            out_arrs = sharded(*concat_in, *zeros)
            jax.block_until_ready(out_arrs)
            times.append(_time.perf_counter() - t0)
        results = [
            {
                name: np.asarray(out_arrs[i]).reshape(n_cores, *out_avals[i].shape)[cid]
                for i, name in enumerate(out_names)
            }
            for cid in range(n_cores)
        ]
        return results, times

    return run


def run_cores(in_maps, dims, n_iters=1):
    key = (dims, N_CORES)
    if key not in _RUNNER_CACHE:
        _RUNNER_CACHE[key] = _make_runner(dims, N_CORES)
    return _RUNNER_CACHE[key](in_maps, n_iters)


def assemble(results, dims):
    b_loc, c, h = dims
    spike = np.concatenate(
        [r["spikeT"].transpose(2, 0, 1).reshape(b_loc, c) for r in results], axis=0
    ).astype(np.float32)
    vout = np.concatenate(
        [r["voutT"].transpose(2, 0, 1).reshape(b_loc, c) for r in results], axis=0
    ).astype(np.float32)
    return spike, vout


def kernel(**inputs):
    in_maps, dims = prep_inputs(**inputs)
    res = run_cores(in_maps, dims, trace=False)
    return assemble(res.results, dims)
